# revision 8
# baseline (speedup 1.0000x reference)
"""Trainium2 Bass kernel for agent attention (sparse_attention problem).

Per-core work (data-parallel over batch B=8 across 8 NeuronCores):
  x[b] [256, 64, 64] -> qkv 3x3 conv (dif-conv + BN folded into weights)
  -> agent attention (8 heads, d=32, 64 agent tokens)
  -> depthwise 3x3 pe conv on v -> 1x1 proj.

Precision strategy (validated numerically against the reference):
  - q,k conv: fp8 e4m3 with DoubleRow perf mode (2 k-tiles of 128
    contracted per instruction -> 2x MAC rate). Weights pre-scaled x32,
    x pre-scaled x16, psum drained with x(1/512). q/k errors only
    perturb softmax logits (measured ~1e-3 final error).
  - v conv: bf16 (v errors pass straight to the output; fp8 there
    costs 3.8e-2 final error - too much).
  - all attention/proj matmuls: bf16 operands, f32 psum.

Engine layout: PE does conv + attention matmuls; transposes go through
the DMA xbar (SP queue); exp/drains on ACT; pe depthwise conv split
DVE/Pool; q,k drains split ACT/Pool; normalization on DVE.
"""
import numpy as np
import ml_dtypes

NUM_HEADS = 8
AGENT_NUM = 64
THETA = 0.7
C = 256
H = W = 64
HW = H * W
D = C // NUM_HEADS          # 32
PS = 8                      # pool size
N_CORES = 8
B = 8

_cache = {}


def _build():
    import concourse.bass as bass
    import concourse.tile as tile
    from concourse import bacc, mybir

    f32 = mybir.dt.float32
    bf16 = mybir.dt.bfloat16
    f8 = mybir.dt.float8e4
    AF = mybir.ActivationFunctionType
    ALU = mybir.AluOpType
    AX = mybir.AxisListType
    DR = mybir.MatmulPerfMode.DoubleRow

    nc = bacc.Bacc("TRN2", target_bir_lowering=False, debug=False,
                   enable_asserts=True, num_devices=N_CORES)

    # x pre-padded to 66x66 on host, bf16
    X = nc.dram_tensor("x", [2, 128, 66 * 66], bf16, kind="ExternalInput").ap()
    # q,k weights: [mc, p, s, t, o] fp8 (pre-scaled x32)
    WQK = nc.dram_tensor("wqk", [4, 128, 9, 2, 128], f8,
                         kind="ExternalInput").ap()
    # v weights: [mc, p, kc, s, o] bf16
    WV = nc.dram_tensor("wv", [2, 128, 2, 9, 128], bf16,
                        kind="ExternalInput").ap()
    BQ = nc.dram_tensor("bq", [128, 6], f32, kind="ExternalInput").ap()
    PEW = nc.dram_tensor("pew", [128, 2, 9], f32, kind="ExternalInput").ap()
    PW = nc.dram_tensor("pw", [128, 2 * 256], bf16, kind="ExternalInput").ap()
    PB = nc.dram_tensor("pb", [128, 2], f32, kind="ExternalInput").ap()
    OUT = nc.dram_tensor("out", [2, 128, HW], f32, kind="ExternalOutput").ap()

    # softmax exp scale: d^-0.5, with the 1/64 agent-pool mean folded in
    SCALE = (D ** -0.5) / (PS * PS)
    QK_DESCALE = 1.0 / 512.0       # undo the x32 weight / x16 input scaling

    with tile.TileContext(nc) as tc:
        from contextlib import ExitStack
        with ExitStack() as top:
            pers = top.enter_context(tc.tile_pool(name="pers", bufs=1))
            q_sb = [pers.tile([128, HW], bf16, tag=f"q{i}", name=f"q{i}")
                    for i in range(2)]
            k_sb = [pers.tile([128, HW], bf16, tag=f"k{i}", name=f"k{i}")
                    for i in range(2)]
            v_pad = [pers.tile([128, 66 * 66], bf16, tag=f"vp{i}",
                               name=f"vp{i}") for i in range(2)]
            v_flat = [pers.tile([128, HW], bf16, tag=f"vf{i}", name=f"vf{i}")
                      for i in range(2)]
            att_out = [pers.tile([128, HW], bf16, tag=f"ao{i}", name=f"ao{i}")
                       for i in range(2)]
            x_bf = pers.tile([128, 2 * 66 * 66], bf16, tag="xbf", name="xbf")
            x8 = pers.tile([128, 2 * 66 * 66], f8, tag="x8", name="x8")
            bq = pers.tile([128, 6], f32, tag="bq", name="bq")
            pew = pers.tile([128, 2, 9], f32, tag="pew", name="pew")
            asum_t = pers.tile([128, 128], f32, tag="asum", name="asum")
            a_sum = [asum_t[:, 64 * i:64 * (i + 1)] for i in range(2)]
            abd_t = pers.tile([128, 512], bf16, tag="abd", name="abd")
            a_bd4 = [abd_t[:, 256 * i:256 * (i + 1)] for i in range(2)]
            az_t = pers.tile([128, 4 * 68], bf16, tag="az", name="az")
            attnZ = [az_t[:, 68 * i:68 * i + 66] for i in range(4)]
            pw = pers.tile([128, 2 * 256], bf16, tag="pw", name="pwt")
            pb = pers.tile([128, 2], f32, tag="pb", name="pbt")

            # x first on the sync DMA queue (conv start gates on it),
            # split into row-halves for earlier compute start
            xbf_v = x_bf[:].rearrange("p (t rc) -> p t rc", t=2, rc=66 * 66)
            x8_v = x8[:].rearrange("p (t rc) -> p t rc", t=2, rc=66 * 66)
            for kc in range(2):
                nc.sync.dma_start(xbf_v[:, kc, :33 * 66], X[kc, :, :33 * 66])
                nc.sync.dma_start(xbf_v[:, kc, 33 * 66:], X[kc, :, 33 * 66:])
            # consts + weights on the gpsimd DMA queue
            nc.gpsimd.dma_start(bq[:], BQ[:])
            nc.gpsimd.dma_start(pew[:], PEW[:])
            nc.gpsimd.dma_start(pw[:], PW[:])
            nc.gpsimd.dma_start(pb[:], PB[:])
            pwv = pw[:].rearrange("p (a b) -> p a b", a=2, b=256)

            # fp8 copy of x, pre-scaled by 16 for e4m3 range use
            for kc in range(2):
                nc.scalar.activation(x8_v[:, kc, :33 * 66],
                                     xbf_v[:, kc, :33 * 66],
                                     AF.Copy, scale=16.0)
                nc.scalar.activation(x8_v[:, kc, 33 * 66:],
                                     xbf_v[:, kc, 33 * 66:],
                                     AF.Copy, scale=16.0)
            # padded window views for the conv
            xbf_w = x_bf[:].rearrange("p (t r c) -> p t r c", t=2, r=66, c=66)
            x8_w = x8[:].rearrange("p (t r c) -> p t r c", t=2, r=66, c=66)

            for cc in range(2):
                vv = v_pad[cc][:].rearrange("p (r c) -> p r c", r=66, c=66)
                nc.gpsimd.memset(vv[:, 0:1, :], 0.0)
                nc.gpsimd.memset(vv[:, 65:66, :], 0.0)
                nc.gpsimd.memset(vv[:, :, 0:1], 0.0)
                nc.gpsimd.memset(vv[:, :, 65:66], 0.0)

            # B-phase pools (top-scope SBUF so they overlap the conv)
            s1sb = top.enter_context(tc.tile_pool(name="s1sb", bufs=2))

            with ExitStack() as ph:
                wpool = ph.enter_context(tc.tile_pool(name="wq", bufs=1))
                cps = ph.enter_context(
                    tc.tile_pool(name="cps", bufs=3, space="PSUM"))

                def conv_group_v(mc):
                    """v conv chunk mc in {0,1}: bf16, 18 matmuls/rowblock."""
                    wt = wpool.tile([128, 2, 9, 128], bf16, tag="wv",
                                    name="wv", bufs=2)
                    nc.gpsimd.dma_start(wt[:], WV[mc])
                    for rb in range(8):
                        ps_t = cps.tile([128, 512], f32, tag="cps",
                                        name="cpst")
                        psv = ps_t[:].rearrange("p (r c) -> p r c", r=8, c=64)
                        i = 0
                        for kc in range(2):
                            for s in range(9):
                                ky, kx = s // 3, s % 3
                                rhs = xbf_w[:, kc, 8 * rb + ky: 8 * rb + ky
                                            + 8, kx: kx + 64]
                                nc.tensor.matmul(
                                    psv, wt[:, kc, s], rhs,
                                    start=(i == 0), stop=(i == 17))
                                i += 1
                        bias = bq[:, 4 + mc: 5 + mc]
                        vv = v_pad[mc][:].rearrange(
                            "p (r c) -> p r c", r=66, c=66)
                        dst = vv[:, 8 * rb + 1: 8 * rb + 9, 1:65]
                        nc.scalar.activation(dst, psv, AF.Identity,
                                             bias=bias)

                def conv_group_qk(mc):
                    """q/k conv chunk mc in {0..3}: fp8 DoubleRow."""
                    wt = wpool.tile([128, 9, 2, 128], f8, tag="wqk",
                                    name="wqk", bufs=4)
                    nc.gpsimd.dma_start(wt[:], WQK[mc])
                    for rb in range(8):
                        ps_t = cps.tile([128, 512], f32, tag="cps",
                                        name="cpst")
                        for s in range(9):
                            ky, kx = s // 3, s % 3
                            rhs = x8_w[:, :, 8 * rb + ky: 8 * rb + ky + 8,
                                       kx: kx + 64]
                            nc.tensor.matmul(
                                ps_t[:], wt[:, s], rhs,
                                start=(s == 0), stop=(s == 8),
                                perf_mode=DR)
                        bias = bq[:, mc: mc + 1]
                        if mc < 2:
                            dst = q_sb[mc][:, 512 * rb: 512 * (rb + 1)]
                        else:
                            dst = k_sb[mc - 2][:, 512 * rb: 512 * (rb + 1)]
                        # split drains between ACT and DVE
                        if rb % 4 == 3:
                            nc.vector.tensor_scalar(dst, ps_t[:], QK_DESCALE,
                                                    bias, ALU.mult, ALU.add)
                        else:
                            nc.scalar.activation(dst, ps_t[:], AF.Identity,
                                                 bias=bias, scale=QK_DESCALE)

                # v first
                conv_group_v(0)
                conv_group_v(1)

                # flat contiguous v copies (DMA-transpose needs 2D src)
                for cc in range(2):
                    vv = v_pad[cc][:].rearrange(
                        "p (r c) -> p r c", r=66, c=66)
                    vfv = v_flat[cc][:].rearrange(
                        "p (r c) -> p r c", r=64, c=64)
                    nc.vector.tensor_copy(vfv, vv[:, 1:65, 1:65])

                # transposed v (positions on partitions) for stage-1 agg,
                # via the DMA transpose xbar on the sync queue
                vts = [None] * 32

                def make_vt(ch):
                    vtc = s1sb.tile([128, 264], bf16, tag="vt", name="vt",
                                    bufs=32)
                    vts[ch] = vtc
                    vtv = vtc[:].rearrange("p (a b) -> p a b", a=4, b=66)
                    nc.gpsimd.memset(vtv[:, :, 64:66], 1.0)
                    for cc in range(2):
                        # xbar transpose needs a 128-col-aligned dst; stage
                        # then split into the two 66-stride blocks
                        vst = s1sb.tile([128, 128], bf16, tag="vst",
                                        name="vst", bufs=4)
                        nc.sync.dma_start_transpose(
                            vst[:], v_flat[cc][:, 128 * ch:128 * (ch + 1)])
                        nc.vector.tensor_copy(
                            vtc[:, (2 * cc) * 66:(2 * cc) * 66 + 64],
                            vst[:, 0:64])
                        nc.vector.tensor_copy(
                            vtc[:, (2 * cc + 1) * 66:(2 * cc + 1) * 66 + 64],
                            vst[:, 64:128])

                for ch in range(32):
                    make_vt(ch)

                # pe depthwise conv, accumulating into att_out (bf16);
                # chains split DVE / Pool
                def pe_conv(cc, g, eng):
                    vvf = v_pad[cc][:].rearrange(
                        "p (r c) -> p r c", r=66, c=66)
                    aof = att_out[cc][:].rearrange(
                        "p (r c) -> p r c", r=64, c=64)
                    r0 = 16 * g
                    dst = aof[:, r0:r0 + 16, :]
                    for s in range(9):
                        ky, kx = s // 3, s % 3
                        sv = vvf[:, r0 + ky: r0 + ky + 16, kx: kx + 64]
                        if s == 0:
                            eng.tensor_scalar_mul(dst, sv, pew[:, cc, 0:1])
                        else:
                            eng.scalar_tensor_tensor(
                                dst, sv, pew[:, cc, s:s + 1], dst,
                                ALU.mult, ALU.add)

                # q
                conv_group_qk(0)
                pe_conv(0, 0, nc.vector)
                pe_conv(0, 1, nc.vector)
                conv_group_qk(1)
                pe_conv(0, 2, nc.vector)
                pe_conv(0, 3, nc.vector)

                # pooling + block-diag a (overlap the k conv)
                for ccq in range(2):
                    qv = q_sb[ccq][:].rearrange(
                        "p (by dy bx dx) -> p by bx dy dx",
                        by=8, dy=8, bx=8, dx=8)
                    nc.vector.tensor_reduce(a_sum[ccq], qv, AX.XY, ALU.add)
                nc.gpsimd.memset(abd_t[:], 0.0)
                for cc in range(2):
                    for j in range(4):
                        nc.vector.tensor_copy(
                            a_bd4[cc][32 * j:32 * j + 32,
                                      64 * j:64 * j + 64],
                            a_sum[cc][32 * j:32 * j + 32, :])

                # k
                conv_group_qk(2)
                pe_conv(1, 0, nc.vector)
                pe_conv(1, 1, nc.vector)
                conv_group_qk(3)
                pe_conv(1, 2, nc.vector)
                pe_conv(1, 3, nc.vector)

            # ---- stage 1 ----
            with ExitStack() as ph:
                st_ps = ph.enter_context(
                    tc.tile_pool(name="stps", bufs=2, space="PSUM"))
                at_ps = ph.enter_context(
                    tc.tile_pool(name="atps", bufs=4, space="PSUM"))
                # attn_pair[cc*2+half]: [128 agents, 132] - only the
                # half-th 66-block is valid (paired rhs keeps LD count low)
                attn_ps = [at_ps.tile([128, 132], f32, tag="at", name="at")
                           for _ in range(4)]
                for ch in range(32):
                    sp = st_ps.tile([128, 512], f32, tag="st", name="stt")
                    for cc in range(2):
                        nc.tensor.matmul(sp[:, 256 * cc:256 * (cc + 1)],
                                         k_sb[cc][:, 128 * ch:128 * (ch + 1)],
                                         a_bd4[cc][:], start=True, stop=True,
                                         skip_group_check=True)
                    et = s1sb.tile([128, 512], bf16, tag="et", name="et")
                    nc.scalar.activation(et[:], sp[:], AF.Exp, scale=SCALE)
                    for cc in range(2):
                        for half in range(2):
                            hp = 2 * cc + half
                            nc.tensor.matmul(
                                attn_ps[hp][:],
                                et[:, 256 * cc + 128 * half:
                                   256 * cc + 128 * (half + 1)],
                                vts[ch][:, 132 * cc:132 * cc + 132],
                                start=(ch == 0), stop=(ch == 31))

                # normalize stage-1 rows by Z1, build attnZ (+ones cols)
                nc.gpsimd.memset(az_t[:], 0.0)
                for hp in range(4):
                    half = hp % 2
                    ap = attn_ps[hp][:, 66 * half:66 * half + 66]
                    r1 = s1sb.tile([128, 1], f32, tag="r1", name="r1")
                    nc.vector.reciprocal(r1[:], ap[:, 64:65])
                    nc.gpsimd.memset(attnZ[hp][0:64, 64:65], 1.0)
                    nc.gpsimd.memset(attnZ[hp][64:128, 65:66], 1.0)
                    nc.vector.tensor_scalar_mul(
                        attnZ[hp][0:64, 0:32], ap[0:64, 0:32], r1[0:64, :])
                    nc.vector.tensor_scalar_mul(
                        attnZ[hp][64:128, 32:64], ap[64:128, 32:64],
                        r1[64:128, :])

            # ---- stage 2 + proj ----
            with ExitStack() as ph:
                s2sb = ph.enter_context(tc.tile_pool(name="s2sb", bufs=3))
                osb = ph.enter_context(tc.tile_pool(name="osb", bufs=3))
                s2_ps = ph.enter_context(
                    tc.tile_pool(name="s2ps", bufs=3, space="PSUM"))
                g_ps = ph.enter_context(
                    tc.tile_pool(name="gps", bufs=3, space="PSUM"))
                pr_ps = ph.enter_context(
                    tc.tile_pool(name="prps", bufs=2, space="PSUM"))

                for nt in range(8):
                    for cc in range(2):
                        tst = s2sb.tile([128, 512], bf16, tag="tst",
                                        name="tst")
                        for half in range(2):
                            hp = 2 * cc + half
                            sp = s2_ps.tile([128, 512], f32, tag="s2",
                                            name="s2t")
                            nc.tensor.matmul(
                                sp[:],
                                a_bd4[cc][:, 128 * half:128 * (half + 1)],
                                q_sb[cc][:, 512 * nt:512 * (nt + 1)],
                                start=True, stop=True)
                            e2 = s2sb.tile([128, 512], bf16, tag="e2",
                                           name="e2")
                            nc.scalar.activation(e2[:], sp[:], AF.Exp,
                                                 scale=SCALE)
                            # 4 transposed-agg matmuls into one psum tile
                            gp = g_ps.tile([128, 272], f32,
                                           tag="g", name="gt")
                            for sub in range(4):
                                nc.tensor.matmul(
                                    gp[:, 68 * sub:68 * sub + 66],
                                    e2[:, 128 * sub:128 * (sub + 1)],
                                    attnZ[hp], start=True, stop=True,
                                    skip_group_check=True)
                            r2 = s2sb.tile([128, 8], f32, tag="r2",
                                           name="r2")
                            gz = gp[:].rearrange(
                                "p (a b) -> p a b", a=4, b=68)[:, :, 64:66]
                            nc.vector.reciprocal(r2[:], gz)
                            if half == 0:
                                ress = [s2sb.tile([128, 128], bf16,
                                                  tag=f"res{i}", name="res")
                                        for i in range(4)]
                            for sub in range(4):
                                sA = r2[:, 2 * sub:2 * sub + 1]
                                sB = r2[:, 2 * sub + 1:2 * sub + 2]
                                inA = gp[:, 68 * sub:68 * sub + 32]
                                inB = gp[:, 68 * sub + 32:68 * sub + 64]
                                oA = ress[sub][:, 64 * half:64 * half + 32]
                                oB = ress[sub][:,
                                               64 * half + 32:64 * half + 64]
                                nc.vector.tensor_scalar_mul(oA, inA, sA)
                                nc.vector.tensor_scalar_mul(oB, inB, sB)
                        # transpose res -> [ch, pos] via DMA xbar, then
                        # accumulate onto the pe part in att_out
                        for sub in range(4):
                            nc.sync.dma_start_transpose(
                                tst[:, 128 * sub:128 * (sub + 1)],
                                ress[sub][:])
                        sl = att_out[cc][:, 512 * nt:512 * (nt + 1)]
                        nc.vector.tensor_tensor(sl, tst[:], sl, ALU.add)
                    for mc in range(2):
                        pp = pr_ps.tile([128, 512], f32, tag="tp", name="prt")
                        for kc in range(2):
                            nc.tensor.matmul(
                                pp[:], pwv[:, kc, 128 * mc:128 * (mc + 1)],
                                att_out[kc][:, 512 * nt:512 * (nt + 1)],
                                start=(kc == 0), stop=(kc == 1))
                        ot = osb.tile([128, 512], f32, tag="ot", name="ott")
                        nc.scalar.activation(ot[:], pp[:], AF.Identity,
                                             bias=pb[:, mc:mc + 1])
                        nc.gpsimd.dma_start(
                            OUT[mc, :, 512 * nt:512 * (nt + 1)], ot[:])

    nc.compile()
    return nc


def _prep_consts(qkv_w, qkv_s, qkv_b, pe_w, pe_s, pe_b, proj_w, proj_s,
                 proj_b):
    f = np.float32
    bf = ml_dtypes.bfloat16
    f8 = ml_dtypes.float8_e4m3
    w = np.asarray(qkv_w, f).copy()          # [768, 256, 3, 3]
    dif = (w[:, :, 0, 1] + w[:, :, 1, 0] + w[:, :, 1, 1] + w[:, :, 1, 2]
           + w[:, :, 2, 1])
    w[:, :, 1, 1] -= THETA * dif
    w *= np.asarray(qkv_s, f)[:, None, None, None]
    w9 = w.reshape(768, 256, 9)

    # q,k: WQK[mc, p, s, t, o] = 32 * w[128*mc+o, 128*t+p, s], fp8
    wqk = w9[:512].reshape(4, 128, 2, 128, 9)        # [mc, o, t, p, s]
    wqk = np.ascontiguousarray(wqk.transpose(0, 3, 4, 2, 1))  # [mc,p,s,t,o]
    wqk = (wqk * 32.0).astype(f8)

    # v: WV[mc, p, kc, s, o] = w[512+128*mc+o, 128*kc+p, s], bf16
    wv = w9[512:].reshape(2, 128, 2, 128, 9)         # [mc, o, kc, p, s]
    wv = np.ascontiguousarray(wv.transpose(0, 3, 2, 4, 1)).astype(bf)

    bqm = np.ascontiguousarray(np.asarray(qkv_b, f).reshape(6, 128).T)

    pe_wf = np.asarray(pe_w, f)[:, 0] * np.asarray(pe_s, f)[:, None, None]
    pew = np.zeros((128, 2, 9), f)
    for kc in range(2):
        for s in range(9):
            pew[:, kc, s] = pe_wf[128 * kc:128 * (kc + 1), s // 3, s % 3]

    pwm = np.asarray(proj_w, f)[:, :, 0, 0] * np.asarray(proj_s, f)[:, None]
    pw = np.ascontiguousarray(
        pwm.T.reshape(2, 128, 256).transpose(1, 0, 2).reshape(128, 512)
    ).astype(bf)
    pbv = np.asarray(proj_b, f) + pwm @ np.asarray(pe_b, f)
    pb = np.ascontiguousarray(pbv.reshape(2, 128).T)

    return dict(wqk=wqk, wv=wv, bq=bqm, pew=pew, pw=pw, pb=pb)


def kernel(x, qkv_w, qkv_s, qkv_b, pe_w, pe_s, pe_b, proj_w, proj_s, proj_b):
    from concourse.bass_utils import run_bass_kernel_spmd

    if "nc" not in _cache:
        _cache["nc"] = _build()
    nc = _cache["nc"]

    consts = _prep_consts(qkv_w, qkv_s, qkv_b, pe_w, pe_s, pe_b, proj_w,
                          proj_s, proj_b)
    bf = ml_dtypes.bfloat16
    x = np.asarray(x, np.float32)
    xp = np.zeros((B, 2, 128, 66, 66), bf)
    xp[:, :, :, 1:65, 1:65] = x.reshape(B, 2, 128, 64, 64).astype(bf)
    xp = xp.reshape(B, 2, 128, 66 * 66)
    in_maps = []
    for b in range(B):
        m = dict(consts)
        m["x"] = np.ascontiguousarray(xp[b])
        in_maps.append(m)

    res = run_bass_kernel_spmd(nc, in_maps, list(range(N_CORES)), trace=False)
    out = np.empty((B, C, H, W), np.float32)
    for b in range(B):
        out[b] = res.results[b]["out"].reshape(C, H, W)
    return out


# revision 11
# speedup vs baseline: 1.1286x; 1.1286x over previous
"""Trainium2 Bass kernel for agent attention (sparse_attention problem).

Per-core work (data-parallel over batch B=8 across 8 NeuronCores):
  x[b] [256, 64, 64] -> qkv 3x3 conv (dif-conv + BN folded into weights)
  -> agent attention (8 heads, d=32, 64 agent tokens)
  -> depthwise 3x3 pe conv on v -> 1x1 proj.

Precision strategy (validated numerically against the reference):
  - q,k conv: fp8 e4m3 with DoubleRow perf mode (2 k-tiles of 128
    contracted per instruction -> 2x MAC rate). Weights pre-scaled x32,
    x pre-scaled x16, psum drained with x(1/512). q/k errors only
    perturb softmax logits (measured ~1e-3 final error).
  - v conv: bf16 (v errors pass straight to the output; fp8 there
    costs 3.8e-2 final error - too much).
  - all attention/proj matmuls: bf16 operands, f32 psum.

Engine layout: PE does conv + attention matmuls; transposes go through
the DMA xbar (SP queue); exp/drains on ACT; pe depthwise conv split
DVE/Pool; q,k drains split ACT/Pool; normalization on DVE.
"""
import numpy as np
import ml_dtypes

NUM_HEADS = 8
AGENT_NUM = 64
THETA = 0.7
C = 256
H = W = 64
HW = H * W
D = C // NUM_HEADS          # 32
PS = 8                      # pool size
N_CORES = 8
B = 8

_cache = {}


def _build():
    import concourse.bass as bass
    import concourse.tile as tile
    from concourse import bacc, mybir

    f32 = mybir.dt.float32
    bf16 = mybir.dt.bfloat16
    f8 = mybir.dt.float8e4
    AF = mybir.ActivationFunctionType
    ALU = mybir.AluOpType
    AX = mybir.AxisListType
    DR = mybir.MatmulPerfMode.DoubleRow

    nc = bacc.Bacc("TRN2", target_bir_lowering=False, debug=False,
                   enable_asserts=True, num_devices=N_CORES)

    # x pre-padded to 66x66 on host, bf16
    X = nc.dram_tensor("x", [2, 128, 66 * 66], bf16, kind="ExternalInput").ap()
    # q,k weights: [mc, p, s, t, o] fp8 (pre-scaled x32)
    WQK = nc.dram_tensor("wqk", [4, 128, 9, 2, 128], f8,
                         kind="ExternalInput").ap()
    # v weights: [mc, p, kc, s, o] bf16
    WV = nc.dram_tensor("wv", [2, 128, 2, 9, 128], bf16,
                        kind="ExternalInput").ap()
    BQ = nc.dram_tensor("bq", [128, 6], f32, kind="ExternalInput").ap()
    PEW = nc.dram_tensor("pew", [128, 2, 9], f32, kind="ExternalInput").ap()
    PW = nc.dram_tensor("pw", [128, 2 * 256], bf16, kind="ExternalInput").ap()
    PB = nc.dram_tensor("pb", [128, 2], f32, kind="ExternalInput").ap()
    IDN = nc.dram_tensor("idn", [128, 128], bf16, kind="ExternalInput").ap()
    OUT = nc.dram_tensor("out", [2, 128, HW], f32, kind="ExternalOutput").ap()

    # softmax exp scale: d^-0.5, with the 1/64 agent-pool mean folded in
    SCALE = (D ** -0.5) / (PS * PS)
    QK_DESCALE = 1.0 / 512.0       # undo the x32 weight / x16 input scaling

    with tile.TileContext(nc) as tc:
        from contextlib import ExitStack
        with ExitStack() as top:
            pers = top.enter_context(tc.tile_pool(name="pers", bufs=1))
            q_sb = [pers.tile([128, HW], bf16, tag=f"q{i}", name=f"q{i}")
                    for i in range(2)]
            k_sb = [pers.tile([128, HW], bf16, tag=f"k{i}", name=f"k{i}")
                    for i in range(2)]
            v_pad = [pers.tile([128, 66 * 66], bf16, tag=f"vp{i}",
                               name=f"vp{i}") for i in range(2)]
            v_flat = [pers.tile([128, HW], bf16, tag=f"vf{i}", name=f"vf{i}")
                      for i in range(2)]
            att_out = [pers.tile([128, HW], bf16, tag=f"ao{i}", name=f"ao{i}")
                       for i in range(2)]
            x_bf = pers.tile([128, 2 * 66 * 66], bf16, tag="xbf", name="xbf")
            x8 = pers.tile([128, 2 * 66 * 66], f8, tag="x8", name="x8")
            bq = pers.tile([128, 6], f32, tag="bq", name="bq")
            pew = pers.tile([128, 2, 9], f32, tag="pew", name="pew")
            pewb = pers.tile([128, 2, 9], bf16, tag="pewb", name="pewb")
            asum_t = pers.tile([128, 128], f32, tag="asum", name="asum")
            a_sum = [asum_t[:, 64 * i:64 * (i + 1)] for i in range(2)]
            abd_t = pers.tile([128, 512], bf16, tag="abd", name="abd")
            a_bd4 = [abd_t[:, 256 * i:256 * (i + 1)] for i in range(2)]
            az_t = pers.tile([128, 4 * 68], bf16, tag="az", name="az")
            attnZ = [az_t[:, 68 * i:68 * i + 66] for i in range(4)]
            pw = pers.tile([128, 2 * 256], bf16, tag="pw", name="pwt")
            pb = pers.tile([128, 2], f32, tag="pb", name="pbt")
            idn = pers.tile([128, 128], bf16, tag="idn", name="idn")

            # x first on the sync DMA queue (conv start gates on it),
            # split into row-halves for earlier compute start
            xbf_v = x_bf[:].rearrange("p (t rc) -> p t rc", t=2, rc=66 * 66)
            x8_v = x8[:].rearrange("p (t rc) -> p t rc", t=2, rc=66 * 66)
            for kc in range(2):
                nc.sync.dma_start(xbf_v[:, kc, :33 * 66], X[kc, :, :33 * 66])
                nc.sync.dma_start(xbf_v[:, kc, 33 * 66:], X[kc, :, 33 * 66:])
            # consts + weights on the gpsimd DMA queue
            nc.gpsimd.dma_start(bq[:], BQ[:])
            nc.gpsimd.dma_start(pew[:], PEW[:])
            nc.gpsimd.dma_start(pw[:], PW[:])
            nc.gpsimd.dma_start(pb[:], PB[:])
            nc.gpsimd.dma_start(idn[:], IDN[:])
            pwv = pw[:].rearrange("p (a b) -> p a b", a=2, b=256)
            nc.vector.tensor_copy(pewb[:], pew[:])

            # fp8 copy of x, pre-scaled by 16 for e4m3 range use
            for kc in range(2):
                nc.scalar.activation(x8_v[:, kc, :33 * 66],
                                     xbf_v[:, kc, :33 * 66],
                                     AF.Copy, scale=16.0)
                nc.scalar.activation(x8_v[:, kc, 33 * 66:],
                                     xbf_v[:, kc, 33 * 66:],
                                     AF.Copy, scale=16.0)
            # padded window views for the conv
            xbf_w = x_bf[:].rearrange("p (t r c) -> p t r c", t=2, r=66, c=66)
            x8_w = x8[:].rearrange("p (t r c) -> p t r c", t=2, r=66, c=66)

            for cc in range(2):
                vv = v_pad[cc][:].rearrange("p (r c) -> p r c", r=66, c=66)
                nc.gpsimd.memset(vv[:, 0:1, :], 0.0)
                nc.gpsimd.memset(vv[:, 65:66, :], 0.0)
                nc.gpsimd.memset(vv[:, :, 0:1], 0.0)
                nc.gpsimd.memset(vv[:, :, 65:66], 0.0)

            # B-phase pools (top-scope SBUF so they overlap the conv)
            s1sb = top.enter_context(tc.tile_pool(name="s1sb", bufs=2))

            with ExitStack() as ph:
                wpool = ph.enter_context(tc.tile_pool(name="wq", bufs=1))
                cps = ph.enter_context(
                    tc.tile_pool(name="cps", bufs=3, space="PSUM"))

                def conv_group_v(mc):
                    """v conv chunk mc in {0,1}: bf16, 18 matmuls/rowblock."""
                    wt = wpool.tile([128, 2, 9, 128], bf16, tag="wv",
                                    name="wv", bufs=2)
                    nc.gpsimd.dma_start(wt[:], WV[mc])
                    for rb in range(8):
                        ps_t = cps.tile([128, 512], f32, tag="cps",
                                        name="cpst")
                        psv = ps_t[:].rearrange("p (r c) -> p r c", r=8, c=64)
                        i = 0
                        for kc in range(2):
                            for s in range(9):
                                ky, kx = s // 3, s % 3
                                rhs = xbf_w[:, kc, 8 * rb + ky: 8 * rb + ky
                                            + 8, kx: kx + 64]
                                nc.tensor.matmul(
                                    psv, wt[:, kc, s], rhs,
                                    start=(i == 0), stop=(i == 17))
                                i += 1
                        bias = bq[:, 4 + mc: 5 + mc]
                        vv = v_pad[mc][:].rearrange(
                            "p (r c) -> p r c", r=66, c=66)
                        dst = vv[:, 8 * rb + 1: 8 * rb + 9, 1:65]
                        nc.scalar.activation(dst, psv, AF.Identity,
                                             bias=bias)

                def conv_group_qk(mc):
                    """q/k conv chunk mc in {0..3}: fp8 DoubleRow."""
                    wt = wpool.tile([128, 9, 2, 128], f8, tag="wqk",
                                    name="wqk", bufs=4)
                    nc.gpsimd.dma_start(wt[:], WQK[mc])
                    for rb in range(8):
                        ps_t = cps.tile([128, 512], f32, tag="cps",
                                        name="cpst")
                        for s in range(9):
                            ky, kx = s // 3, s % 3
                            rhs = x8_w[:, :, 8 * rb + ky: 8 * rb + ky + 8,
                                       kx: kx + 64]
                            nc.tensor.matmul(
                                ps_t[:], wt[:, s], rhs,
                                start=(s == 0), stop=(s == 8),
                                perf_mode=DR)
                        bias = bq[:, mc: mc + 1]
                        if mc < 2:
                            dst = q_sb[mc][:, 512 * rb: 512 * (rb + 1)]
                        else:
                            dst = k_sb[mc - 2][:, 512 * rb: 512 * (rb + 1)]
                        # split drains between ACT and DVE
                        if rb % 4 == 3:
                            nc.vector.tensor_scalar(dst, ps_t[:], QK_DESCALE,
                                                    bias, ALU.mult, ALU.add)
                        else:
                            nc.scalar.activation(dst, ps_t[:], AF.Identity,
                                                 bias=bias, scale=QK_DESCALE)

                # v first
                conv_group_v(0)
                conv_group_v(1)

                # flat contiguous v copies (DMA-transpose needs 2D src)
                for cc in range(2):
                    vv = v_pad[cc][:].rearrange(
                        "p (r c) -> p r c", r=66, c=66)
                    vfv = v_flat[cc][:].rearrange(
                        "p (r c) -> p r c", r=64, c=64)
                    nc.vector.tensor_copy(vfv, vv[:, 1:65, 1:65])

                # transposed v (positions on partitions) for stage-1 agg,
                # via the DMA transpose xbar on the sync queue
                vts = [None] * 32

                def make_vt(ch):
                    vtc = s1sb.tile([128, 264], bf16, tag="vt", name="vt",
                                    bufs=32)
                    vts[ch] = vtc
                    vtv = vtc[:].rearrange("p (a b) -> p a b", a=4, b=66)
                    nc.gpsimd.memset(vtv[:, :, 64:66], 1.0)
                    for cc in range(2):
                        # xbar transpose needs a 128-col-aligned dst; stage
                        # then split into the two 66-stride blocks
                        vst = s1sb.tile([128, 128], bf16, tag="vst",
                                        name="vst", bufs=4)
                        nc.sync.dma_start_transpose(
                            vst[:], v_flat[cc][:, 128 * ch:128 * (ch + 1)])
                        nc.vector.tensor_copy(
                            vtc[:, (2 * cc) * 66:(2 * cc) * 66 + 64],
                            vst[:, 0:64])
                        nc.vector.tensor_copy(
                            vtc[:, (2 * cc + 1) * 66:(2 * cc + 1) * 66 + 64],
                            vst[:, 64:128])

                for ch in range(32):
                    make_vt(ch)

                # pe depthwise conv, accumulating into att_out (bf16);
                # chains split DVE / Pool
                def pe_conv(cc, g, eng):
                    vvf = v_pad[cc][:].rearrange(
                        "p (r c) -> p r c", r=66, c=66)
                    aof = att_out[cc][:].rearrange(
                        "p (r c) -> p r c", r=64, c=64)
                    r0 = 16 * g
                    dst = aof[:, r0:r0 + 16, :]
                    tmp = s1sb.tile([128, 16, 64], bf16, tag="petmp",
                                    name="petmp", bufs=2)
                    for s in range(9):
                        ky, kx = s // 3, s % 3
                        sv = vvf[:, r0 + ky: r0 + ky + 16, kx: kx + 64]
                        wb = pewb[:, cc, s:s + 1].to_broadcast([128, 16, 64])
                        if s == 0:
                            eng.tensor_tensor(dst, sv, wb, ALU.mult)
                        else:
                            eng.tensor_tensor(tmp[:], sv, wb, ALU.mult)
                            eng.tensor_tensor(dst, tmp[:], dst, ALU.add)

                # q
                conv_group_qk(0)
                pe_conv(0, 0, nc.vector)
                pe_conv(0, 1, nc.vector)
                conv_group_qk(1)
                pe_conv(0, 2, nc.vector)
                pe_conv(0, 3, nc.vector)

                # pooling + block-diag a (overlap the k conv)
                for ccq in range(2):
                    qv = q_sb[ccq][:].rearrange(
                        "p (by dy bx dx) -> p by bx dy dx",
                        by=8, dy=8, bx=8, dx=8)
                    nc.vector.tensor_reduce(a_sum[ccq], qv, AX.XY, ALU.add)
                nc.gpsimd.memset(abd_t[:], 0.0)
                for cc in range(2):
                    for j in range(4):
                        nc.vector.tensor_copy(
                            a_bd4[cc][32 * j:32 * j + 32,
                                      64 * j:64 * j + 64],
                            a_sum[cc][32 * j:32 * j + 32, :])

                # k
                conv_group_qk(2)
                pe_conv(1, 0, nc.vector)
                pe_conv(1, 1, nc.vector)
                conv_group_qk(3)
                pe_conv(1, 2, nc.vector)
                pe_conv(1, 3, nc.vector)

            # ---- stage 1 ----
            with ExitStack() as ph:
                st_ps = ph.enter_context(
                    tc.tile_pool(name="stps", bufs=2, space="PSUM"))
                at_ps = ph.enter_context(
                    tc.tile_pool(name="atps", bufs=4, space="PSUM"))
                # attn_pair[cc*2+half]: [128 agents, 132] - only the
                # half-th 66-block is valid (paired rhs keeps LD count low)
                attn_ps = [at_ps.tile([128, 132], f32, tag="at", name="at")
                           for _ in range(4)]
                for chp in range(16):
                    sp = st_ps.tile([128, 1024], f32, tag="st", name="stt")
                    for p in range(2):
                        ch = 2 * chp + p
                        for cc in range(2):
                            nc.tensor.matmul(
                                sp[:, 512 * p + 256 * cc:
                                   512 * p + 256 * (cc + 1)],
                                k_sb[cc][:, 128 * ch:128 * (ch + 1)],
                                a_bd4[cc][:], start=True, stop=True,
                                skip_group_check=True)
                    et = s1sb.tile([128, 1024], bf16, tag="et", name="et")
                    nc.scalar.activation(et[:], sp[:], AF.Exp, scale=SCALE)
                    for p in range(2):
                        ch = 2 * chp + p
                        for cc in range(2):
                            for half in range(2):
                                hp = 2 * cc + half
                                nc.tensor.matmul(
                                    attn_ps[hp][:],
                                    et[:, 512 * p + 256 * cc + 128 * half:
                                       512 * p + 256 * cc
                                       + 128 * (half + 1)],
                                    vts[ch][:, 132 * cc:132 * cc + 132],
                                    start=(ch == 0), stop=(ch == 31))

                # normalize stage-1 rows by Z1, build attnZ (+ones cols)
                nc.gpsimd.memset(az_t[:], 0.0)
                for hp in range(4):
                    half = hp % 2
                    ap = attn_ps[hp][:, 66 * half:66 * half + 66]
                    r1 = s1sb.tile([128, 1], f32, tag="r1", name="r1")
                    nc.vector.reciprocal(r1[:], ap[:, 64:65])
                    nc.gpsimd.memset(attnZ[hp][0:64, 64:65], 1.0)
                    nc.gpsimd.memset(attnZ[hp][64:128, 65:66], 1.0)
                    nc.vector.tensor_scalar_mul(
                        attnZ[hp][0:64, 0:32], ap[0:64, 0:32], r1[0:64, :])
                    nc.vector.tensor_scalar_mul(
                        attnZ[hp][64:128, 32:64], ap[64:128, 32:64],
                        r1[64:128, :])

            # ---- stage 2 + proj ----
            with ExitStack() as ph:
                s2sb = ph.enter_context(tc.tile_pool(name="s2sb", bufs=3))
                osb = ph.enter_context(tc.tile_pool(name="osb", bufs=3))
                s2_ps = ph.enter_context(
                    tc.tile_pool(name="s2ps", bufs=3, space="PSUM"))
                g_ps = ph.enter_context(
                    tc.tile_pool(name="gps", bufs=2, space="PSUM"))
                t_ps = ph.enter_context(
                    tc.tile_pool(name="tps", bufs=2, space="PSUM"))
                pr_ps = ph.enter_context(
                    tc.tile_pool(name="prps", bufs=1, space="PSUM"))

                for nt in range(8):
                    for cc in range(2):
                        res = s2sb.tile([128, 512], bf16, tag="res",
                                        name="res")
                        for half in range(2):
                            hp = 2 * cc + half
                            sp = s2_ps.tile([128, 512], f32, tag="s2",
                                            name="s2t")
                            nc.tensor.matmul(
                                sp[:],
                                a_bd4[cc][:, 128 * half:128 * (half + 1)],
                                q_sb[cc][:, 512 * nt:512 * (nt + 1)],
                                start=True, stop=True)
                            e2 = s2sb.tile([128, 512], bf16, tag="e2",
                                           name="e2")
                            nc.scalar.activation(e2[:], sp[:], AF.Exp,
                                                 scale=SCALE)
                            # 4 transposed-agg matmuls into one psum tile
                            gp = g_ps.tile([128, 272], f32,
                                           tag="g", name="gt")
                            for sub in range(4):
                                nc.tensor.matmul(
                                    gp[:, 68 * sub:68 * sub + 66],
                                    e2[:, 128 * sub:128 * (sub + 1)],
                                    attnZ[hp], start=True, stop=True,
                                    skip_group_check=True)
                            r2 = s2sb.tile([128, 8], f32, tag="r2",
                                           name="r2")
                            gz = gp[:].rearrange(
                                "p (a b) -> p a b", a=4, b=68)[:, :, 64:66]
                            nc.vector.reciprocal(r2[:], gz)
                            # single normalize op: res[., sub, half, g, j]
                            # = gp[., sub, g, j] * r2[., sub, g] (bcast j)
                            gv = gp[:].rearrange(
                                "p (a b) -> p a b", a=4,
                                b=68)[:, :, 0:64].rearrange(
                                "p a (g j) -> p a g j", g=2, j=32)
                            rv = r2[:].rearrange(
                                "p (a g) -> p a g", a=4,
                                g=2).to_broadcast([128, 4, 2, 32])
                            ov = res[:].rearrange(
                                "p (a h g j) -> p a h g j", a=4, h=2,
                                g=2, j=32)[:, :, half]
                            nc.vector.tensor_tensor(ov, gv, rv, ALU.mult)
                        # transpose res -> [ch, pos] on PE, accumulate
                        # onto the pe part in att_out
                        tp = t_ps.tile([128, 512], bf16, tag="tp",
                                       name="tpt")
                        for sub in range(4):
                            nc.tensor.transpose(
                                tp[:, 128 * sub:128 * (sub + 1)],
                                res[:, 128 * sub:128 * (sub + 1)], idn[:])
                        sl = att_out[cc][:, 512 * nt:512 * (nt + 1)]
                        nc.vector.tensor_tensor(sl, tp[:], sl, ALU.add)
                    for mc in range(2):
                        pp = pr_ps.tile([128, 512], f32, tag="tp", name="prt")
                        for kc in range(2):
                            nc.tensor.matmul(
                                pp[:], pwv[:, kc, 128 * mc:128 * (mc + 1)],
                                att_out[kc][:, 512 * nt:512 * (nt + 1)],
                                start=(kc == 0), stop=(kc == 1))
                        ot = osb.tile([128, 512], f32, tag="ot", name="ott")
                        nc.scalar.activation(ot[:], pp[:], AF.Identity,
                                             bias=pb[:, mc:mc + 1])
                        nc.gpsimd.dma_start(
                            OUT[mc, :, 512 * nt:512 * (nt + 1)], ot[:])

    nc.compile()
    return nc


def _prep_consts(qkv_w, qkv_s, qkv_b, pe_w, pe_s, pe_b, proj_w, proj_s,
                 proj_b):
    f = np.float32
    bf = ml_dtypes.bfloat16
    f8 = ml_dtypes.float8_e4m3
    w = np.asarray(qkv_w, f).copy()          # [768, 256, 3, 3]
    dif = (w[:, :, 0, 1] + w[:, :, 1, 0] + w[:, :, 1, 1] + w[:, :, 1, 2]
           + w[:, :, 2, 1])
    w[:, :, 1, 1] -= THETA * dif
    w *= np.asarray(qkv_s, f)[:, None, None, None]
    w9 = w.reshape(768, 256, 9)

    # q,k: WQK[mc, p, s, t, o] = 32 * w[128*mc+o, 128*t+p, s], fp8
    wqk = w9[:512].reshape(4, 128, 2, 128, 9)        # [mc, o, t, p, s]
    wqk = np.ascontiguousarray(wqk.transpose(0, 3, 4, 2, 1))  # [mc,p,s,t,o]
    wqk = (wqk * 32.0).astype(f8)

    # v: WV[mc, p, kc, s, o] = w[512+128*mc+o, 128*kc+p, s], bf16
    wv = w9[512:].reshape(2, 128, 2, 128, 9)         # [mc, o, kc, p, s]
    wv = np.ascontiguousarray(wv.transpose(0, 3, 2, 4, 1)).astype(bf)

    bqm = np.ascontiguousarray(np.asarray(qkv_b, f).reshape(6, 128).T)

    pe_wf = np.asarray(pe_w, f)[:, 0] * np.asarray(pe_s, f)[:, None, None]
    pew = np.zeros((128, 2, 9), f)
    for kc in range(2):
        for s in range(9):
            pew[:, kc, s] = pe_wf[128 * kc:128 * (kc + 1), s // 3, s % 3]

    pwm = np.asarray(proj_w, f)[:, :, 0, 0] * np.asarray(proj_s, f)[:, None]
    pw = np.ascontiguousarray(
        pwm.T.reshape(2, 128, 256).transpose(1, 0, 2).reshape(128, 512)
    ).astype(bf)
    pbv = np.asarray(proj_b, f) + pwm @ np.asarray(pe_b, f)
    pb = np.ascontiguousarray(pbv.reshape(2, 128).T)

    idn = np.eye(128, dtype=f).astype(bf)
    return dict(wqk=wqk, wv=wv, bq=bqm, pew=pew, pw=pw, pb=pb, idn=idn)


def kernel(x, qkv_w, qkv_s, qkv_b, pe_w, pe_s, pe_b, proj_w, proj_s, proj_b):
    from concourse.bass_utils import run_bass_kernel_spmd

    if "nc" not in _cache:
        _cache["nc"] = _build()
    nc = _cache["nc"]

    consts = _prep_consts(qkv_w, qkv_s, qkv_b, pe_w, pe_s, pe_b, proj_w,
                          proj_s, proj_b)
    bf = ml_dtypes.bfloat16
    x = np.asarray(x, np.float32)
    xp = np.zeros((B, 2, 128, 66, 66), bf)
    xp[:, :, :, 1:65, 1:65] = x.reshape(B, 2, 128, 64, 64).astype(bf)
    xp = xp.reshape(B, 2, 128, 66 * 66)
    in_maps = []
    for b in range(B):
        m = dict(consts)
        m["x"] = np.ascontiguousarray(xp[b])
        in_maps.append(m)

    res = run_bass_kernel_spmd(nc, in_maps, list(range(N_CORES)), trace=False)
    out = np.empty((B, C, H, W), np.float32)
    for b in range(B):
        out[b] = res.results[b]["out"].reshape(C, H, W)
    return out


# revision 15
# speedup vs baseline: 1.5716x; 1.3926x over previous
"""Trainium2 Bass kernel for agent attention (sparse_attention problem).

Per-core work (data-parallel over batch B=8 across 8 NeuronCores):
  x[b] [256, 64, 64] -> qkv 3x3 conv (dif-conv + BN folded into weights)
  -> agent attention (8 heads, d=32, 64 agent tokens)
  -> depthwise 3x3 pe conv on v -> 1x1 proj.

Precision strategy (validated numerically against the reference):
  - q,k conv: fp8 e4m3 with DoubleRow perf mode (2 k-tiles of 128
    contracted per instruction -> 2x MAC rate). Weights pre-scaled x32,
    x pre-scaled x16, psum drained with x(1/512). q/k errors only
    perturb softmax logits (measured ~1e-3 final error).
  - v conv: bf16 (v errors pass straight to the output; fp8 there
    costs 3.8e-2 final error - too much).
  - all attention/proj matmuls: bf16 operands, f32 psum.

Engine layout: PE does conv + attention matmuls; transposes go through
the DMA xbar (SP queue); exp/drains on ACT; pe depthwise conv split
DVE/Pool; q,k drains split ACT/Pool; normalization on DVE.
"""
import numpy as np
import ml_dtypes

NUM_HEADS = 8
AGENT_NUM = 64
THETA = 0.7
C = 256
H = W = 64
HW = H * W
D = C // NUM_HEADS          # 32
PS = 8                      # pool size
N_CORES = 8
B = 8

_cache = {}


def _build():
    import concourse.bass as bass
    import concourse.tile as tile
    from concourse import bacc, mybir

    f32 = mybir.dt.float32
    bf16 = mybir.dt.bfloat16
    f8 = mybir.dt.float8e4
    AF = mybir.ActivationFunctionType
    ALU = mybir.AluOpType
    AX = mybir.AxisListType
    DR = mybir.MatmulPerfMode.DoubleRow

    nc = bacc.Bacc("TRN2", target_bir_lowering=False, debug=False,
                   enable_asserts=True, num_devices=N_CORES)

    # x pre-padded to 66x66 on host, bf16
    X = nc.dram_tensor("x", [2, 128, 66 * 66], bf16, kind="ExternalInput").ap()
    # q,k weights: [mc, p, s, t, o] fp8 (pre-scaled x32)
    WQK = nc.dram_tensor("wqk", [4, 128, 9, 2, 128], f8,
                         kind="ExternalInput").ap()
    # v weights: [mc, p, kc, s, o] bf16
    WV = nc.dram_tensor("wv", [2, 128, 2, 9, 128], bf16,
                        kind="ExternalInput").ap()
    BQ = nc.dram_tensor("bq", [128, 6], f32, kind="ExternalInput").ap()
    PEW = nc.dram_tensor("pew", [128, 2, 9], f32, kind="ExternalInput").ap()
    PW = nc.dram_tensor("pw", [128, 2 * 256], bf16, kind="ExternalInput").ap()
    PB = nc.dram_tensor("pb", [128, 2], f32, kind="ExternalInput").ap()
    IDN = nc.dram_tensor("idn", [128, 128], bf16, kind="ExternalInput").ap()
    OUT = nc.dram_tensor("out", [2, 128, HW], f32, kind="ExternalOutput").ap()

    # softmax exp scale: d^-0.5, with the 1/64 agent-pool mean folded in
    SCALE = (D ** -0.5) / (PS * PS)
    QK_DESCALE = 1.0 / 512.0       # undo the x32 weight / x16 input scaling

    with tile.TileContext(nc) as tc:
        from contextlib import ExitStack
        with ExitStack() as top:
            pers = top.enter_context(tc.tile_pool(name="pers", bufs=1))
            q_sb = [pers.tile([128, HW], bf16, tag=f"q{i}", name=f"q{i}")
                    for i in range(2)]
            k_sb = [pers.tile([128, HW], bf16, tag=f"k{i}", name=f"k{i}")
                    for i in range(2)]
            v_pad = [pers.tile([128, 66 * 66], bf16, tag=f"vp{i}",
                               name=f"vp{i}") for i in range(2)]
            vts_t = pers.tile([128, 32 * 264], bf16, tag="vts", name="vts")
            att_out = [pers.tile([128, HW], bf16, tag=f"ao{i}", name=f"ao{i}")
                       for i in range(2)]
            x_bf = pers.tile([128, 2 * 66 * 66], bf16, tag="xbf", name="xbf")
            x8 = pers.tile([128, 2 * 66 * 66], f8, tag="x8", name="x8")
            bq = pers.tile([128, 6], f32, tag="bq", name="bq")
            pew = pers.tile([128, 2, 9], f32, tag="pew", name="pew")
            asum_t = pers.tile([128, 128], f32, tag="asum", name="asum")
            a_sum = [asum_t[:, 64 * i:64 * (i + 1)] for i in range(2)]
            abd_t = pers.tile([128, 512], bf16, tag="abd", name="abd")
            a_bd4 = [abd_t[:, 256 * i:256 * (i + 1)] for i in range(2)]
            az_t = pers.tile([128, 4 * 68], bf16, tag="az", name="az")
            attnZ = [az_t[:, 68 * i:68 * i + 66] for i in range(4)]
            pw = pers.tile([128, 2 * 256], bf16, tag="pw", name="pwt")
            pb = pers.tile([128, 2], f32, tag="pb", name="pbt")
            idn = pers.tile([128, 128], bf16, tag="idn", name="idn")

            # x first on the sync DMA queue (conv start gates on it),
            # split into row-halves for earlier compute start
            xbf_v = x_bf[:].rearrange("p (t rc) -> p t rc", t=2, rc=66 * 66)
            x8_v = x8[:].rearrange("p (t rc) -> p t rc", t=2, rc=66 * 66)
            for kc in range(2):
                nc.sync.dma_start(xbf_v[:, kc, :33 * 66], X[kc, :, :33 * 66])
                nc.sync.dma_start(xbf_v[:, kc, 33 * 66:], X[kc, :, 33 * 66:])
            # bias needed by the first v drain - first on the gpsimd queue
            nc.gpsimd.dma_start(bq[:], BQ[:])
            pwv = pw[:].rearrange("p (a b) -> p a b", a=2, b=256)

            # fp8 copy of x, pre-scaled by 16 for e4m3 range use
            for kc in range(2):
                nc.vector.tensor_scalar_mul(
                    x8_v[:, kc, :33 * 66], xbf_v[:, kc, :33 * 66], 16.0)
                nc.vector.tensor_scalar_mul(
                    x8_v[:, kc, 33 * 66:], xbf_v[:, kc, 33 * 66:], 16.0)
            # padded window views for the conv
            xbf_w = x_bf[:].rearrange("p (t r c) -> p t r c", t=2, r=66, c=66)
            x8_w = x8[:].rearrange("p (t r c) -> p t r c", t=2, r=66, c=66)

            vts_ones = vts_t[:].rearrange("p (ch a b) -> p ch a b",
                                          ch=32, a=4, b=66)[:, :, :, 64:66]
            nc.gpsimd.memset(vts_ones, 1.0)
            for cc in range(2):
                vv = v_pad[cc][:].rearrange("p (r c) -> p r c", r=66, c=66)
                nc.gpsimd.memset(vv[:, 0:1, :], 0.0)
                nc.gpsimd.memset(vv[:, 65:66, :], 0.0)
                nc.gpsimd.memset(vv[:, :, 0:1], 0.0)
                nc.gpsimd.memset(vv[:, :, 65:66], 0.0)

            # B-phase pools (top-scope SBUF so they overlap the conv)
            s1sb = top.enter_context(tc.tile_pool(name="s1sb", bufs=2))

            with ExitStack() as ph:
                wpool = ph.enter_context(tc.tile_pool(name="wq", bufs=1))
                cps = ph.enter_context(
                    tc.tile_pool(name="cps", bufs=3, space="PSUM"))

                def conv_group_v(mc):
                    """v conv chunk mc in {0,1}: bf16, 18 matmuls/rowblock."""
                    wt = wpool.tile([128, 2, 9, 128], bf16, tag="wv",
                                    name="wv", bufs=2)
                    nc.gpsimd.dma_start(wt[:], WV[mc])
                    for rb in range(8):
                        ps_t = cps.tile([128, 512], f32, tag="cps",
                                        name="cpst")
                        psv = ps_t[:].rearrange("p (r c) -> p r c", r=8, c=64)
                        i = 0
                        for kc in range(2):
                            for s in range(9):
                                ky, kx = s // 3, s % 3
                                rhs = xbf_w[:, kc, 8 * rb + ky: 8 * rb + ky
                                            + 8, kx: kx + 64]
                                nc.tensor.matmul(
                                    psv, wt[:, kc, s], rhs,
                                    start=(i == 0), stop=(i == 17))
                                i += 1
                        bias = bq[:, 4 + mc: 5 + mc]
                        vv = v_pad[mc][:].rearrange(
                            "p (r c) -> p r c", r=66, c=66)
                        dst = vv[:, 8 * rb + 1: 8 * rb + 9, 1:65]
                        nc.scalar.activation(dst, psv, AF.Identity,
                                             bias=bias)

                def conv_group_qk(mc):
                    """q/k conv chunk mc in {0..3}: fp8 DoubleRow."""
                    wt = wpool.tile([128, 9, 2, 128], f8, tag="wqk",
                                    name="wqk", bufs=4)
                    nc.gpsimd.dma_start(wt[:], WQK[mc])
                    for rb in range(8):
                        ps_t = cps.tile([128, 512], f32, tag="cps",
                                        name="cpst")
                        for s in range(9):
                            ky, kx = s // 3, s % 3
                            rhs = x8_w[:, :, 8 * rb + ky: 8 * rb + ky + 8,
                                       kx: kx + 64]
                            nc.tensor.matmul(
                                ps_t[:], wt[:, s], rhs,
                                start=(s == 0), stop=(s == 8),
                                perf_mode=DR)
                        bias = bq[:, mc: mc + 1]
                        if mc < 2:
                            dst = q_sb[mc][:, 512 * rb: 512 * (rb + 1)]
                        else:
                            dst = k_sb[mc - 2][:, 512 * rb: 512 * (rb + 1)]
                        # split drains between ACT and DVE
                        if rb % 4 == 3:
                            nc.vector.tensor_scalar(dst, ps_t[:], QK_DESCALE,
                                                    bias, ALU.mult, ALU.add)
                        else:
                            nc.scalar.activation(dst, ps_t[:], AF.Identity,
                                                 bias=bias, scale=QK_DESCALE)

                # v first
                conv_group_v(0)
                nc.gpsimd.dma_start(pew[:], PEW[:])
                nc.gpsimd.dma_start(idn[:], IDN[:])
                conv_group_v(1)
                nc.gpsimd.dma_start(pw[:], PW[:])
                nc.gpsimd.dma_start(pb[:], PB[:])

                # transposed v (positions on partitions) for stage-1 agg
                vts = [vts_t[:, 264 * ch:264 * (ch + 1)] for ch in range(32)]
                tr_ps = ph.enter_context(
                    tc.tile_pool(name="trps", bufs=2, space="PSUM"))

                def make_vt(ch):
                    vtc = vts[ch]
                    for cc in range(2):
                        vv = v_pad[cc][:].rearrange(
                            "p (r c) -> p r c", r=66, c=66)
                        vstg = s1sb.tile([128, 128], bf16, tag="vstg",
                                         name="vstg", bufs=4)
                        nc.vector.tensor_copy(
                            vstg[:].rearrange("p (r c) -> p r c", r=2, c=64),
                            vv[:, 2 * ch + 1: 2 * ch + 3, 1:65])
                        tp = tr_ps.tile([128, 128], bf16, tag="tr",
                                        name="trt")
                        nc.tensor.transpose(tp[:], vstg[:], idn[:])
                        dst = vtc[:, 132 * cc:132 * cc + 132].rearrange(
                            "p (a b) -> p a b", a=2, b=66)[:, :, 0:64]
                        sv = tp[:].rearrange("p (a b) -> p a b", a=2, b=64)
                        if (ch + cc) % 2 == 0:
                            nc.vector.tensor_copy(dst, sv)
                        else:
                            nc.scalar.activation(dst, sv, AF.Copy)

                for ch in range(32):
                    make_vt(ch)

                # pe depthwise conv, accumulating into att_out (bf16);
                # chains split DVE / Pool
                def pe_conv(cc, g, eng):
                    vvf = v_pad[cc][:].rearrange(
                        "p (r c) -> p r c", r=66, c=66)
                    aof = att_out[cc][:].rearrange(
                        "p (r c) -> p r c", r=64, c=64)
                    r0 = 16 * g
                    dst = aof[:, r0:r0 + 16, :]
                    for s in range(9):
                        ky, kx = s // 3, s % 3
                        sv = vvf[:, r0 + ky: r0 + ky + 16, kx: kx + 64]
                        if s == 0:
                            eng.tensor_scalar_mul(dst, sv, pew[:, cc, 0:1])
                        else:
                            eng.scalar_tensor_tensor(
                                dst, sv, pew[:, cc, s:s + 1], dst,
                                ALU.mult, ALU.add)

                # q
                conv_group_qk(0)
                pe_conv(0, 0, nc.vector)
                pe_conv(0, 1, nc.vector)
                conv_group_qk(1)
                pe_conv(0, 2, nc.vector)
                pe_conv(0, 3, nc.vector)

                # pooling + block-diag a (overlap the k conv)
                for ccq in range(2):
                    pool1 = s1sb.tile([128, 512], f32, tag="pool1",
                                      name="pool1", bufs=2)
                    qv = q_sb[ccq][:].rearrange(
                        "p (r dx) -> p r dx", r=512, dx=8)
                    nc.vector.tensor_reduce(pool1[:], qv, AX.X, ALU.add)
                    p1v = pool1[:].rearrange(
                        "p (by dy bx) -> p by bx dy", by=8, dy=8, bx=8)
                    nc.vector.tensor_reduce(a_sum[ccq], p1v, AX.X, ALU.add)
                nc.gpsimd.memset(abd_t[:], 0.0)
                for cc in range(2):
                    for j in range(4):
                        nc.vector.tensor_copy(
                            a_bd4[cc][32 * j:32 * j + 32,
                                      64 * j:64 * j + 64],
                            a_sum[cc][32 * j:32 * j + 32, :])

                # k
                conv_group_qk(2)
                pe_conv(1, 0, nc.vector)
                pe_conv(1, 1, nc.vector)
                conv_group_qk(3)
                pe_conv(1, 2, nc.vector)
                pe_conv(1, 3, nc.vector)

            # ---- stage 1 ----
            with ExitStack() as ph:
                st_ps = ph.enter_context(
                    tc.tile_pool(name="stps", bufs=2, space="PSUM"))
                at_ps = ph.enter_context(
                    tc.tile_pool(name="atps", bufs=4, space="PSUM"))
                attn_ps = [at_ps.tile([128, 66], f32, tag="at", name="at")
                           for _ in range(4)]
                for chp in range(16):
                    sp = st_ps.tile([128, 1024], f32, tag="st", name="stt")
                    for p in range(2):
                        ch = 2 * chp + p
                        for cc in range(2):
                            nc.tensor.matmul(
                                sp[:, 512 * p + 256 * cc:
                                   512 * p + 256 * (cc + 1)],
                                k_sb[cc][:, 128 * ch:128 * (ch + 1)],
                                a_bd4[cc][:], start=True, stop=True,
                                skip_group_check=True)
                    et = s1sb.tile([128, 1024], bf16, tag="et", name="et")
                    nc.scalar.activation(et[:], sp[:], AF.Exp, scale=SCALE)
                    for p in range(2):
                        ch = 2 * chp + p
                        for cc in range(2):
                            for half in range(2):
                                hp = 2 * cc + half
                                nc.tensor.matmul(
                                    attn_ps[hp][:],
                                    et[:, 512 * p + 256 * cc + 128 * half:
                                       512 * p + 256 * cc
                                       + 128 * (half + 1)],
                                    vts[ch][:, 66 * hp:66 * hp + 66],
                                    start=(ch == 0), stop=(ch == 31))

                # normalize stage-1 rows by Z1, build attnZ (+ones cols)
                nc.gpsimd.memset(az_t[:], 0.0)
                for hp in range(4):
                    ap = attn_ps[hp][:]
                    r1 = s1sb.tile([128, 1], f32, tag="r1", name="r1")
                    nc.vector.reciprocal(r1[:], ap[:, 64:65])
                    nc.gpsimd.memset(attnZ[hp][0:64, 64:65], 1.0)
                    nc.gpsimd.memset(attnZ[hp][64:128, 65:66], 1.0)
                    nc.vector.tensor_scalar_mul(
                        attnZ[hp][0:64, 0:32], ap[0:64, 0:32], r1[0:64, :])
                    nc.vector.tensor_scalar_mul(
                        attnZ[hp][64:128, 32:64], ap[64:128, 32:64],
                        r1[64:128, :])

            # ---- stage 2 + proj ----
            with ExitStack() as ph:
                s2sb = ph.enter_context(tc.tile_pool(name="s2sb", bufs=3))
                osb = ph.enter_context(tc.tile_pool(name="osb", bufs=3))
                s2_ps = ph.enter_context(
                    tc.tile_pool(name="s2ps", bufs=3, space="PSUM"))
                g_ps = ph.enter_context(
                    tc.tile_pool(name="gps", bufs=2, space="PSUM"))
                t_ps = ph.enter_context(
                    tc.tile_pool(name="tps", bufs=2, space="PSUM"))
                pr_ps = ph.enter_context(
                    tc.tile_pool(name="prps", bufs=1, space="PSUM"))

                for nt in range(8):
                    for cc in range(2):
                        res = s2sb.tile([128, 512], bf16, tag="res",
                                        name="res")
                        for half in range(2):
                            hp = 2 * cc + half
                            sp = s2_ps.tile([128, 512], f32, tag="s2",
                                            name="s2t")
                            nc.tensor.matmul(
                                sp[:],
                                a_bd4[cc][:, 128 * half:128 * (half + 1)],
                                q_sb[cc][:, 512 * nt:512 * (nt + 1)],
                                start=True, stop=True)
                            e2 = s2sb.tile([128, 512], bf16, tag="e2",
                                           name="e2")
                            nc.scalar.activation(e2[:], sp[:], AF.Exp,
                                                 scale=SCALE)
                            # 4 transposed-agg matmuls into one psum tile
                            gp = g_ps.tile([128, 272], f32,
                                           tag="g", name="gt")
                            for sub in range(4):
                                nc.tensor.matmul(
                                    gp[:, 68 * sub:68 * sub + 66],
                                    e2[:, 128 * sub:128 * (sub + 1)],
                                    attnZ[hp], start=True, stop=True,
                                    skip_group_check=True)
                            r2 = s2sb.tile([128, 8], f32, tag="r2",
                                           name="r2")
                            gz = gp[:].rearrange(
                                "p (a b) -> p a b", a=4, b=68)[:, :, 64:66]
                            nc.vector.reciprocal(r2[:], gz)
                            # single normalize op: res[., sub, half, g, j]
                            # = gp[., sub, g, j] * r2[., sub, g] (bcast j)
                            gv = gp[:].rearrange(
                                "p (a b) -> p a b", a=4,
                                b=68)[:, :, 0:64].rearrange(
                                "p a (g j) -> p a g j", g=2, j=32)
                            rv = r2[:].rearrange(
                                "p (a g) -> p a g", a=4,
                                g=2).to_broadcast([128, 4, 2, 32])
                            ov = res[:].rearrange(
                                "p (a h g j) -> p a h g j", a=4, h=2,
                                g=2, j=32)[:, :, half]
                            nc.vector.tensor_tensor(ov, gv, rv, ALU.mult)
                        # transpose res -> [ch, pos] on PE, accumulate
                        # onto the pe part in att_out
                        tp = t_ps.tile([128, 512], bf16, tag="tp",
                                       name="tpt")
                        for sub in range(4):
                            nc.tensor.transpose(
                                tp[:, 128 * sub:128 * (sub + 1)],
                                res[:, 128 * sub:128 * (sub + 1)], idn[:])
                        sl = att_out[cc][:, 512 * nt:512 * (nt + 1)]
                        nc.vector.tensor_tensor(sl, tp[:], sl, ALU.add)
                    for mc in range(2):
                        pp = pr_ps.tile([128, 512], f32, tag="tp", name="prt")
                        for kc in range(2):
                            nc.tensor.matmul(
                                pp[:], pwv[:, kc, 128 * mc:128 * (mc + 1)],
                                att_out[kc][:, 512 * nt:512 * (nt + 1)],
                                start=(kc == 0), stop=(kc == 1))
                        ot = osb.tile([128, 512], f32, tag="ot", name="ott")
                        nc.scalar.activation(ot[:], pp[:], AF.Identity,
                                             bias=pb[:, mc:mc + 1])
                        nc.gpsimd.dma_start(
                            OUT[mc, :, 512 * nt:512 * (nt + 1)], ot[:])

    nc.compile()
    return nc


def _prep_consts(qkv_w, qkv_s, qkv_b, pe_w, pe_s, pe_b, proj_w, proj_s,
                 proj_b):
    f = np.float32
    bf = ml_dtypes.bfloat16
    f8 = ml_dtypes.float8_e4m3
    w = np.asarray(qkv_w, f).copy()          # [768, 256, 3, 3]
    dif = (w[:, :, 0, 1] + w[:, :, 1, 0] + w[:, :, 1, 1] + w[:, :, 1, 2]
           + w[:, :, 2, 1])
    w[:, :, 1, 1] -= THETA * dif
    w *= np.asarray(qkv_s, f)[:, None, None, None]
    w9 = w.reshape(768, 256, 9)

    # q,k: WQK[mc, p, s, t, o] = 32 * w[128*mc+o, 128*t+p, s], fp8
    wqk = w9[:512].reshape(4, 128, 2, 128, 9)        # [mc, o, t, p, s]
    wqk = np.ascontiguousarray(wqk.transpose(0, 3, 4, 2, 1))  # [mc,p,s,t,o]
    wqk = (wqk * 32.0).astype(f8)

    # v: WV[mc, p, kc, s, o] = w[512+128*mc+o, 128*kc+p, s], bf16
    wv = w9[512:].reshape(2, 128, 2, 128, 9)         # [mc, o, kc, p, s]
    wv = np.ascontiguousarray(wv.transpose(0, 3, 2, 4, 1)).astype(bf)

    bqm = np.ascontiguousarray(np.asarray(qkv_b, f).reshape(6, 128).T)

    pe_wf = np.asarray(pe_w, f)[:, 0] * np.asarray(pe_s, f)[:, None, None]
    pew = np.zeros((128, 2, 9), f)
    for kc in range(2):
        for s in range(9):
            pew[:, kc, s] = pe_wf[128 * kc:128 * (kc + 1), s // 3, s % 3]

    pwm = np.asarray(proj_w, f)[:, :, 0, 0] * np.asarray(proj_s, f)[:, None]
    pw = np.ascontiguousarray(
        pwm.T.reshape(2, 128, 256).transpose(1, 0, 2).reshape(128, 512)
    ).astype(bf)
    pbv = np.asarray(proj_b, f) + pwm @ np.asarray(pe_b, f)
    pb = np.ascontiguousarray(pbv.reshape(2, 128).T)

    idn = np.eye(128, dtype=f).astype(bf)
    return dict(wqk=wqk, wv=wv, bq=bqm, pew=pew, pw=pw, pb=pb, idn=idn)


def kernel(x, qkv_w, qkv_s, qkv_b, pe_w, pe_s, pe_b, proj_w, proj_s, proj_b):
    from concourse.bass_utils import run_bass_kernel_spmd

    if "nc" not in _cache:
        _cache["nc"] = _build()
    nc = _cache["nc"]

    consts = _prep_consts(qkv_w, qkv_s, qkv_b, pe_w, pe_s, pe_b, proj_w,
                          proj_s, proj_b)
    bf = ml_dtypes.bfloat16
    x = np.asarray(x, np.float32)
    xp = np.zeros((B, 2, 128, 66, 66), bf)
    xp[:, :, :, 1:65, 1:65] = x.reshape(B, 2, 128, 64, 64).astype(bf)
    xp = xp.reshape(B, 2, 128, 66 * 66)
    in_maps = []
    for b in range(B):
        m = dict(consts)
        m["x"] = np.ascontiguousarray(xp[b])
        in_maps.append(m)

    res = run_bass_kernel_spmd(nc, in_maps, list(range(N_CORES)), trace=False)
    out = np.empty((B, C, H, W), np.float32)
    for b in range(B):
        out[b] = res.results[b]["out"].reshape(C, H, W)
    return out
